# revision 9
# baseline (speedup 1.0000x reference)
"""Trainium2 Bass kernel for nn_MultiHeadAttention_64647847739885.

Reference semantics (fp32):
    Wq_eff = softmax(Wq + tril_mask, axis=-2)   (if maskout else Wq)  [H,D,DK]
    Wk_eff = softmax(Wk + tril_mask, axis=-2)   (if maskout else Wk)
    WqQ = einsum('btd,hdk->bhtk', Q, Wq_eff)
    WkK = einsum('bsd,hdk->bhsk', K, Wk_eff)
    WvV = einsum('bsd,hdv->bhsv', V, Wv)
    scores = einsum('bhtk,bhsk->bhts', WqQ, WkK) / sqrt(dk)
    probs = softmax(scores, axis=-2)            # over the QUERY axis t!
    ctx = einsum('bhts,bhsv->bhtv', probs, WvV) -> (B,T,H*DV) @ Wo

Device strategy (8 NeuronCores, SPMD): core c handles batch b = c//2 and
head-group g = c%2 (heads_per_core=8, use_rs=True: pairwise ReduceScatter of
the partial output projection; each core emits its T/2 rows).  Fallback
variant: heads_per_core=16, use_rs=False (redundant pair, full output).
All activations live in transposed layouts so every matmul contracts over the
partition axis with natural tile loads; softmax over the query axis t becomes
a free-axis row softmax of scores^T; all softmax denominators fold into
per-partition scales.  Host does layout-only work (transpose/pack/slice) plus
constant mask/ones generation.
"""

import os
import numpy as np

import concourse.bacc as bacc
import concourse.mybir as mybir
import concourse.tile as tile
from concourse import bass_utils
from concourse.bass_interp import get_hw_module

B, T, D = 4, 1024, 1024
H, DK = 16, 64
P = 128
N_CORES = 8
ND = D // P          # d tiles (contraction for projections)
NS = T // P          # s tiles
NT2 = T // 512       # moving-dim halves

F32 = mybir.dt.float32
BF16 = mybir.dt.bfloat16

RG_PAIRS = [[0, 1], [2, 3], [4, 5], [6, 7]]


def _emit_rep(nc, tc, aps, pp, tp, op_, psb, psc, maskout, HC, use_rs, rep):
    """Emit one full forward pass."""
    NPAIR = HC // 2
    WCOLS = HC * DK
    NWC2 = max(1, WCOLS // 512)
    NMROW = WCOLS // P
    qT, kT, vT, wq, wk, wv, wo, tri, ones, out = aps
    R = f"r{rep}"

    qq = pp.tile([P, NPAIR, T], BF16, tag="qq")
    kk = pp.tile([P, NPAIR, T], BF16, tag="kk")
    wvv = pp.tile([P, NS, WCOLS], BF16, tag="wvv")
    ctx = pp.tile([P, NPAIR, T], BF16, tag="ctx")
    ones_t = pp.tile([P, 1], BF16, tag="ones")
    nc.gpsimd.dma_start(ones_t[:], ones[:])

    # ---------------- Phase V: wvv = (V @ Wv) in (s x v) ----------
    with tc.tile_pool(name=f"s2{R}", bufs=1) as s2:
        vT_t = s2.tile([P, ND, T], BF16, tag="vT")
        wv_t = s2.tile([P, ND, WCOLS], BF16, tag="wv")
        nc.gpsimd.dma_start(vT_t[:], vT.rearrange("(i p) t -> p i t", p=P))
        nc.gpsimd.dma_start(wv_t[:], wv.rearrange("(i p) c -> p i c", p=P))
        for g in range(NWC2):
            gw = min(512, WCOLS)
            for st in range(NS):
                ps = psb.tile([P, 1024], F32, tag="big")
                for i in range(ND):
                    nc.tensor.matmul(
                        ps[:, :gw],
                        lhsT=vT_t[:, i, st * P:(st + 1) * P],
                        rhs=wv_t[:, i, g * 512:g * 512 + gw],
                        start=(i == 0), stop=(i == ND - 1),
                    )
                nc.vector.tensor_copy(
                    wvv[:, st, g * 512:g * 512 + gw], ps[:, :gw])

    # ---------------- Phase W: weight softmax + projections -------
    with tc.tile_pool(name=f"s1{R}", bufs=1) as s1:
        qT_t = s1.tile([P, ND, T], BF16, tag="qT")
        kT_t = s1.tile([P, ND, T], BF16, tag="kT")
        wq_t = s1.tile([P, ND, WCOLS], BF16, tag="wq")
        wk_t = s1.tile([P, ND, WCOLS], BF16, tag="wk")
        nc.gpsimd.dma_start(qT_t[:], qT.rearrange("(i p) t -> p i t", p=P))
        nc.gpsimd.dma_start(kT_t[:], kT.rearrange("(i p) t -> p i t", p=P))
        nc.gpsimd.dma_start(wq_t[:], wq.rearrange("(i p) c -> p i c", p=P))
        nc.gpsimd.dma_start(wk_t[:], wk.rearrange("(i p) c -> p i c", p=P))

        cscale = []  # per-pair (P,1) f32 scale folded into qq, or None
        if maskout:
            tri_t = s1.tile([P, WCOLS], BF16, tag="tri")
            nc.gpsimd.dma_start(tri_t[:], tri[:])
            wqm = s1.tile([P, ND, WCOLS], BF16, tag="wqm")
            wkm = s1.tile([P, ND, WCOLS], BF16, tag="wkm")
            for (src, dst) in ((wq_t, wqm), (wk_t, wkm)):
                for i in range(ND):
                    nc.scalar.activation(
                        dst[:, i, :], src[:, i, :],
                        mybir.ActivationFunctionType.Exp)
                # only d-tile 0 has masked entries (tril on (1024,64))
                nc.vector.tensor_mul(dst[:, 0, :], dst[:, 0, :], tri_t[:])
            wq_eff, wk_eff = wqm, wkm
            # column sums -> per-pair scale c = 1/(sq*sk)
            for p in range(NPAIR):
                ps_q = psb.tile([P, 1024], F32, tag="big")
                ps_k = psb.tile([P, 1024], F32, tag="big")
                for i in range(ND):
                    nc.tensor.matmul(
                        ps_q[:, :1], lhsT=wqm[:, i, p * P:(p + 1) * P],
                        rhs=ones_t[:], start=(i == 0), stop=(i == ND - 1))
                for i in range(ND):
                    nc.tensor.matmul(
                        ps_k[:, :1], lhsT=wkm[:, i, p * P:(p + 1) * P],
                        rhs=ones_t[:], start=(i == 0), stop=(i == ND - 1))
                sq = tp.tile([P, 1], F32, tag="sq")
                sk = tp.tile([P, 1], F32, tag="sk")
                nc.vector.tensor_copy(sq[:], ps_q[:, :1])
                nc.vector.tensor_copy(sk[:], ps_k[:, :1])
                prod = tp.tile([P, 1], F32, tag="prod")
                nc.vector.tensor_mul(prod[:], sq[:], sk[:])
                c = tp.tile([P, 1], F32, tag=f"c{p}")
                nc.vector.reciprocal(c[:], prod[:])
                cscale.append(c)
        else:
            wq_eff, wk_eff = wq_t, wk_t
            cscale = [None] * NPAIR

        # projections: qq/kk (k x t) per head pair
        for p in range(NPAIR):
            ps = psb.tile([P, 1024], F32, tag="big")
            for i in range(ND):
                for n in range(NT2):
                    nc.tensor.matmul(
                        ps[:, n * 512:(n + 1) * 512],
                        lhsT=wq_eff[:, i, p * P:(p + 1) * P],
                        rhs=qT_t[:, i, n * 512:(n + 1) * 512],
                        start=(i == 0), stop=(i == ND - 1))
            if cscale[p] is not None:
                nc.vector.tensor_scalar_mul(qq[:, p, :], ps[:], cscale[p][:])
            else:
                nc.vector.tensor_copy(qq[:, p, :], ps[:])
            ps = psb.tile([P, 1024], F32, tag="big")
            for i in range(ND):
                for n in range(NT2):
                    nc.tensor.matmul(
                        ps[:, n * 512:(n + 1) * 512],
                        lhsT=wk_eff[:, i, p * P:(p + 1) * P],
                        rhs=kT_t[:, i, n * 512:(n + 1) * 512],
                        start=(i == 0), stop=(i == ND - 1))
            nc.vector.tensor_copy(kk[:, p, :], ps[:])

    # ---------------- Phase H: per-head attention ------------------
    for p in range(NPAIR):
        pctx_a = psc.tile([P, T], F32, tag="ctxpA")
        pctx_b = psc.tile([P, T], F32, tag="ctxpB")
        pctx_h = (pctx_a, pctx_b)
        for st in range(NS):
            for half, base in ((0, 0), (1, 64)):
                pctx = pctx_h[half]
                psco = psb.tile([P, 1024], F32, tag="big")
                for n in range(NT2):
                    nc.tensor.matmul(
                        psco[:, n * 512:(n + 1) * 512],
                        lhsT=kk[base:base + 64, p, st * P:(st + 1) * P],
                        rhs=qq[base:base + 64, p, n * 512:(n + 1) * 512],
                        start=True, stop=True,
                        tile_position=(base, 0))
                e = tp.tile([P, T], BF16, tag="e")
                rs = tp.tile([P, 1], F32, tag="rs")
                nc.scalar.activation(
                    e[:], psco[:], mybir.ActivationFunctionType.Exp,
                    scale=0.125, accum_out=rs[:])
                r = tp.tile([P, 1], F32, tag="r")
                nc.vector.reciprocal(r[:], rs[:])
                hcol = (2 * p + half) * DK
                wvs = tp.tile([P, DK], BF16, tag="wvs")
                nc.vector.tensor_scalar_mul(
                    wvs[:], wvv[:, st, hcol:hcol + DK], r[:])
                for n in range(NT2):
                    nc.tensor.matmul(
                        pctx[base:base + 64, n * 512:(n + 1) * 512],
                        lhsT=wvs[:],
                        rhs=e[:, n * 512:(n + 1) * 512],
                        start=(st == 0), stop=(st == NS - 1),
                        tile_position=(0, base))
        nc.vector.tensor_copy(ctx[0:64, p, :], pctx_h[0][0:64, :])
        nc.vector.tensor_copy(ctx[64:128, p, :], pctx_h[1][64:128, :])

    # ---------------- Phase O: output projection -------------------
    with tc.tile_pool(name=f"s3{R}", bufs=1) as s3:
        wo_t = s3.tile([P, NMROW, D], BF16, tag="wo")
        nc.gpsimd.dma_start(wo_t[:], wo.rearrange("(i p) n -> p i n", p=P))
        if use_rs:
            dp_cm = tc.tile_pool(name=f"dram{R}", bufs=1, space="DRAM")
            dp = dp_cm.__enter__()
            obounce = dp.tile([T, D], F32, tag="ob")
            ors = dp.tile([T // 2, D], F32, tag="ors")
        for tt in range(T // P):
            pso = psb.tile([P, 1024], F32, tag="big")
            for m in range(NMROW):
                for n in range(NT2):
                    nc.tensor.matmul(
                        pso[:, n * 512:(n + 1) * 512],
                        lhsT=ctx[:, m, tt * P:(tt + 1) * P],
                        rhs=wo_t[:, m, n * 512:(n + 1) * 512],
                        start=(m == 0), stop=(m == NMROW - 1))
            osb = op_.tile([P, D], F32, tag="o")
            nc.vector.tensor_copy(osb[:], pso[:])
            dst = obounce if use_rs else out
            nc.sync.dma_start(dst[tt * P:(tt + 1) * P, :], osb[:])
        if use_rs:
            nc.gpsimd.collective_compute(
                "ReduceScatter",
                mybir.AluOpType.add,
                replica_groups=RG_PAIRS,
                ins=[obounce.opt()],
                outs=[ors.opt()],
            )
            nc.sync.dma_start(out[:], ors[:])
            dp_cm.__exit__(None, None, None)


def _build(maskout: bool, heads_per_core: int, use_rs: bool, repeat: int = 1):
    """Build + compile the SPMD program. Returns compiled nc."""
    HC = heads_per_core
    WCOLS = HC * DK
    OUT_ROWS = T // 2 if use_rs else T

    nc = bacc.Bacc("TRN2", target_bir_lowering=False, debug=False,
                   num_devices=N_CORES)

    qT = nc.dram_tensor("qT", [D, T], F32, kind="ExternalInput").ap()
    kT = nc.dram_tensor("kT", [D, T], F32, kind="ExternalInput").ap()
    vT = nc.dram_tensor("vT", [D, T], F32, kind="ExternalInput").ap()
    wq = nc.dram_tensor("wq", [D, WCOLS], F32, kind="ExternalInput").ap()
    wk = nc.dram_tensor("wk", [D, WCOLS], F32, kind="ExternalInput").ap()
    wv = nc.dram_tensor("wv", [D, WCOLS], F32, kind="ExternalInput").ap()
    wo = nc.dram_tensor("wo", [WCOLS, D], F32, kind="ExternalInput").ap()
    tri = nc.dram_tensor("tri", [P, WCOLS], F32, kind="ExternalInput").ap()
    ones = nc.dram_tensor("ones", [P, 1], F32, kind="ExternalInput").ap()
    out = nc.dram_tensor("out", [OUT_ROWS, D], F32, kind="ExternalOutput").ap()
    aps = (qT, kT, vT, wq, wk, wv, wo, tri, ones, out)

    with tile.TileContext(nc) as tc:
        with (
            tc.tile_pool(name="persist", bufs=1) as pp,
            tc.tile_pool(name="trans", bufs=4) as tp,
            tc.tile_pool(name="osb", bufs=2) as op_,
            tc.tile_pool(name="psum_big", bufs=2, space="PSUM") as psb,
            tc.tile_pool(name="psum_ctx", bufs=1, space="PSUM") as psc,
        ):
            for rep in range(repeat):
                _emit_rep(nc, tc, aps, pp, tp, op_, psb, psc,
                          maskout, HC, use_rs, rep)

    nc.compile()
    nc.m = get_hw_module(nc.m)
    return nc


_CACHE: dict = {}


def _get_program(maskout: bool, heads_per_core: int, use_rs: bool,
                 repeat: int = 1):
    key = (maskout, heads_per_core, use_rs, repeat)
    if key not in _CACHE:
        _CACHE[key] = _build(*key)
    return _CACHE[key]


def _prep_inputs(Q, K, V, Wq, Wk, Wv, Wo, heads_per_core):
    """Host-side layout-only sharding: per-core input dicts."""
    HC = heads_per_core
    WCOLS = HC * DK
    tri = (np.arange(P)[:, None] >= (np.arange(WCOLS)[None, :] % DK)) \
        .astype(np.float32)
    ones = np.ones((P, 1), np.float32)
    in_maps = []
    for c in range(N_CORES):
        b = c // 2
        if HC == H:
            hsel = np.arange(H)
        else:
            g = c % 2
            hsel = np.arange(g * HC, (g + 1) * HC)
        # (H,D,DK) -> (D, HC*DK) packed columns for selected heads
        wq_p = np.ascontiguousarray(
            Wq[hsel].transpose(1, 0, 2).reshape(D, WCOLS))
        wk_p = np.ascontiguousarray(
            Wk[hsel].transpose(1, 0, 2).reshape(D, WCOLS))
        wv_p = np.ascontiguousarray(
            Wv[hsel].transpose(1, 0, 2).reshape(D, WCOLS))
        wo_p = np.ascontiguousarray(Wo.reshape(H, DK, D)[hsel].reshape(WCOLS, D))
        in_maps.append({
            "qT": np.ascontiguousarray(Q[b].T),
            "kT": np.ascontiguousarray(K[b].T),
            "vT": np.ascontiguousarray(V[b].T),
            "wq": wq_p, "wk": wk_p, "wv": wv_p, "wo": wo_p,
            "tri": tri, "ones": ones,
        })
    return in_maps


def run(Q, K, V, Wq, Wk, Wv, Wo, maskout, heads_per_core=8, use_rs=True,
        repeat=1):
    Q = np.asarray(Q, np.float32)
    K = np.asarray(K, np.float32)
    V = np.asarray(V, np.float32)
    Wq = np.asarray(Wq, np.float32)
    Wk = np.asarray(Wk, np.float32)
    Wv = np.asarray(Wv, np.float32)
    Wo = np.asarray(Wo, np.float32)
    mk = bool(np.asarray(maskout).item())
    nc = _get_program(mk, heads_per_core, use_rs, repeat)
    in_maps = _prep_inputs(Q, K, V, Wq, Wk, Wv, Wo, heads_per_core)
    res = bass_utils.run_bass_kernel_spmd(
        nc, in_maps, list(range(N_CORES)), trace=False)
    outf = np.empty((B, T, D), np.float32)
    for c in range(N_CORES):
        b = c // 2
        if use_rs:
            half = c % 2
            outf[b, half * (T // 2):(half + 1) * (T // 2), :] = \
                res.results[c]["out"]
        else:
            if c % 2 == 0:
                outf[b] = res.results[c]["out"]
    return outf, res


def kernel(Q, K, V, Wq, Wk, Wv, Wo, maskout):
    outf, _ = run(Q, K, V, Wq, Wk, Wv, Wo, maskout,
                  heads_per_core=int(os.environ.get("MHA_HPC", "8")),
                  use_rs=bool(int(os.environ.get("MHA_RS", "1"))))
    return outf


# revision 20
# speedup vs baseline: 1108.7431x; 1108.7431x over previous
"""Trainium2 Bass kernel for nn_MultiHeadAttention_64647847739885.

Reference semantics (fp32):
    Wq_eff = softmax(Wq + tril_mask, axis=-2)   (if maskout else Wq)  [H,D,DK]
    Wk_eff = softmax(Wk + tril_mask, axis=-2)   (if maskout else Wk)
    WqQ = einsum('btd,hdk->bhtk', Q, Wq_eff)
    WkK = einsum('bsd,hdk->bhsk', K, Wk_eff)
    WvV = einsum('bsd,hdv->bhsv', V, Wv)
    scores = einsum('bhtk,bhsk->bhts', WqQ, WkK) / sqrt(dk)
    probs = softmax(scores, axis=-2)            # over the QUERY axis t!
    ctx = einsum('bhts,bhsv->bhtv', probs, WvV) -> (B,T,H*DV) @ Wo

Device strategy (8 NeuronCores, SPMD): core c handles batch b = c//2 and
head-group g = c%2 (heads_per_core=8, use_rs=True: pairwise ReduceScatter of
the partial output projection; each core emits its T/2 rows).  Fallback
variant: heads_per_core=16, use_rs=False (redundant pair, full output).
All activations live in transposed layouts so every matmul contracts over the
partition axis with natural tile loads; softmax over the query axis t becomes
a free-axis row softmax of scores^T; all softmax denominators fold into
per-partition scales.  Host does layout-only work (transpose/pack/slice) plus
constant mask/ones generation.
"""

import os
import numpy as np

import concourse.bacc as bacc
import concourse.mybir as mybir
import concourse.tile as tile
from concourse import bass_utils
from concourse.bass_interp import get_hw_module

B, T, D = 4, 1024, 1024
H, DK = 16, 64
P = 128
N_CORES = 8
ND = D // P          # d tiles (contraction for projections)
NS = T // P          # s tiles
NT2 = T // 512       # moving-dim halves

F32 = mybir.dt.float32
BF16 = mybir.dt.bfloat16

RG_PAIRS = [[0, 1], [2, 3], [4, 5], [6, 7]]


def _emit_rep(nc, tc, aps, pp, tp, op_, psb, psc, maskout, HC, use_rs, rep,
              phases=frozenset({"loads", "weights", "wvv", "proj", "heads", "out"})):
    """Emit one full forward pass (or a phase subset, for timing only)."""
    NPAIR = HC // 2
    WCOLS = HC * DK
    NWC2 = max(1, WCOLS // 512)
    NMROW = WCOLS // P
    qT, kT, vT, wq, wk, wv, wo, tri, ones, out = aps

    qq = pp.tile([P, NPAIR, T], BF16, tag="qq")
    kk = pp.tile([P, NPAIR, T], BF16, tag="kk")
    wvv = pp.tile([P, NS, WCOLS], BF16, tag="wvv")
    ctx = pp.tile([P, NPAIR, T], BF16, tag="ctx")
    ones_t = pp.tile([P, 1], BF16, tag="ones")
    ones_f = pp.tile([P, 1], F32, tag="ones_f")
    qT_t = pp.tile([P, ND, T], BF16, tag="qT")
    kT_t = pp.tile([P, ND, T], BF16, tag="kT")
    vT_t = pp.tile([P, ND, T], BF16, tag="vT")
    wq_t = pp.tile([P, ND, WCOLS], BF16, tag="wq")
    wk_t = pp.tile([P, ND, WCOLS], BF16, tag="wk")
    wv_t = pp.tile([P, ND, WCOLS], BF16, tag="wv")
    wo_t = pp.tile([P, NMROW, D], BF16, tag="wo")
    tri_t = pp.tile([P, WCOLS], BF16, tag="tri")

    # ---- loads: HWDGE f32 into staging, DVE-cast to bf16 ----
    # (SWDGE cast-DMA measured ~137GB/s vs ~690GB/s for HWDGE f32)
    if "loads" in phases:
        nc.gpsimd.dma_start(ones_t[:], ones[:])
        nc.gpsimd.dma_start(ones_f[:], ones[:])
        if maskout:
            stt = tp.tile([P, WCOLS], F32, tag="stt")
            nc.sync.dma_start(stt[:], tri[:])
            nc.vector.tensor_copy(tri_t[:], stt[:])
        for mat_ap, mat_t, nd_, w in (
                (wq, wq_t, ND, WCOLS), (wk, wk_t, ND, WCOLS),
                (vT, vT_t, ND, T), (wv, wv_t, ND, WCOLS),
                (qT, qT_t, ND, T), (kT, kT_t, ND, T),
                (wo, wo_t, NMROW, D)):
            for i in range(nd_):
                st = tp.tile([P, T], F32, tag="st")
                nc.sync.dma_start(st[:, :w], mat_ap[i * P:(i + 1) * P, :])
                nc.vector.tensor_copy(mat_t[:, i, :], st[:, :w])

    # ---------------- weight softmax (exp in place + fold scales) --
    cscale = []  # per-pair (P,1) f32 scale folded into qq, or None
    if maskout and "weights" in phases:
        for w_t in (wq_t, wk_t):
            for i in range(ND):
                nc.scalar.activation(
                    w_t[:, i, :], w_t[:, i, :],
                    mybir.ActivationFunctionType.Exp)
            # only d-tile 0 has masked entries (tril on (1024,64))
            nc.vector.tensor_mul(w_t[:, 0, :], w_t[:, 0, :], tri_t[:])
        # column sums over d via ones-stationary matmuls: (1 x WCOLS)
        sums_sb = []
        for w_t in (wq_t, wk_t):
            ps_s = psb.tile([P, 1024], F32, tag="big")
            for g in range(NWC2):
                gw = min(512, WCOLS)
                for i in range(ND):
                    nc.tensor.matmul(
                        ps_s[:1, g * 512:g * 512 + gw],
                        lhsT=ones_t[:],
                        rhs=w_t[:, i, g * 512:g * 512 + gw],
                        start=(i == 0), stop=(i == ND - 1))
            ssb = tp.tile([1, WCOLS], F32, tag="ssb")
            nc.vector.tensor_copy(ssb[:], ps_s[:1, :WCOLS])
            sums_sb.append(ssb)
        # transpose (1 x 128) slices into (128 x 1) via f32 matmul
        for p in range(NPAIR):
            ps_t = psb.tile([P, 1024], F32, tag="big")
            # two single-shot groups in different PSUM banks (cols 0 / 512)
            nc.tensor.matmul(
                ps_t[:, 0:1], lhsT=sums_sb[0][:, p * P:(p + 1) * P],
                rhs=ones_f[:1, :], start=True, stop=True)
            nc.tensor.matmul(
                ps_t[:, 512:513], lhsT=sums_sb[1][:, p * P:(p + 1) * P],
                rhs=ones_f[:1, :], start=True, stop=True)
            sqv = tp.tile([P, 1], F32, tag="sqv")
            nc.vector.tensor_copy(sqv[:], ps_t[:, 0:1])
            prod = tp.tile([P, 1], F32, tag="prod")
            nc.vector.tensor_mul(prod[:], sqv[:], ps_t[:, 512:513])
            c = tp.tile([P, 1], F32, tag=f"c{p}")
            nc.vector.reciprocal(c[:], prod[:])
            cscale.append(c)
    else:
        cscale = [None] * NPAIR

    # ---------------- wvv = (V @ Wv) in (s x v) --------------------
    for g in range(NWC2 if "wvv" in phases else 0):
        gw = min(512, WCOLS)
        for st in range(NS):
            ps = psb.tile([P, 1024], F32, tag="big")
            for i in range(ND):
                nc.tensor.matmul(
                    ps[:, :gw],
                    lhsT=vT_t[:, i, st * P:(st + 1) * P],
                    rhs=wv_t[:, i, g * 512:g * 512 + gw],
                    start=(i == 0), stop=(i == ND - 1),
                )
            nc.vector.tensor_copy(
                wvv[:, st, g * 512:g * 512 + gw], ps[:, :gw])

    # ---------------- projections: qq/kk (k x t) per head pair -----
    for p in range(NPAIR if "proj" in phases else 0):
        ps = psb.tile([P, 1024], F32, tag="big")
        for i in range(ND):
            for n in range(NT2):
                nc.tensor.matmul(
                    ps[:, n * 512:(n + 1) * 512],
                    lhsT=wq_t[:, i, p * P:(p + 1) * P],
                    rhs=qT_t[:, i, n * 512:(n + 1) * 512],
                    start=(i == 0), stop=(i == ND - 1))
        if cscale[p] is not None:
            nc.vector.tensor_scalar_mul(qq[:, p, :], ps[:], cscale[p][:])
        else:
            nc.vector.tensor_copy(qq[:, p, :], ps[:])
        ps = psb.tile([P, 1024], F32, tag="big")
        for i in range(ND):
            for n in range(NT2):
                nc.tensor.matmul(
                    ps[:, n * 512:(n + 1) * 512],
                    lhsT=wk_t[:, i, p * P:(p + 1) * P],
                    rhs=kT_t[:, i, n * 512:(n + 1) * 512],
                    start=(i == 0), stop=(i == ND - 1))
        nc.vector.tensor_copy(kk[:, p, :], ps[:])

    # ---------------- Phase H: per-head attention ------------------
    for p in range(NPAIR if "heads" in phases else 0):
        pctx_a = psc.tile([P, T], F32, tag="ctxpA")
        pctx_b = psc.tile([P, T], F32, tag="ctxpB")
        pctx_h = (pctx_a, pctx_b)
        for st in range(NS):
            for half, base in ((0, 0), (1, 64)):
                pctx = pctx_h[half]
                psco = psb.tile([P, 1024], F32, tag="big")
                for n in range(NT2):
                    nc.tensor.matmul(
                        psco[:, n * 512:(n + 1) * 512],
                        lhsT=kk[base:base + 64, p, st * P:(st + 1) * P],
                        rhs=qq[base:base + 64, p, n * 512:(n + 1) * 512],
                        start=True, stop=True,
                        tile_position=(base, 0))
                e = tp.tile([P, T], BF16, tag="e")
                rs = tp.tile([P, 1], F32, tag="rs")
                nc.scalar.activation(
                    e[:], psco[:], mybir.ActivationFunctionType.Exp,
                    scale=0.125, accum_out=rs[:])
                r = tp.tile([P, 1], F32, tag="r")
                nc.vector.reciprocal(r[:], rs[:])
                hcol = (2 * p + half) * DK
                wvs = tp.tile([P, DK], BF16, tag="wvs")
                nc.vector.tensor_scalar_mul(
                    wvs[:], wvv[:, st, hcol:hcol + DK], r[:])
                for n in range(NT2):
                    nc.tensor.matmul(
                        pctx[base:base + 64, n * 512:(n + 1) * 512],
                        lhsT=wvs[:],
                        rhs=e[:, n * 512:(n + 1) * 512],
                        start=(st == 0), stop=(st == NS - 1),
                        tile_position=(0, base))
        nc.vector.tensor_copy(ctx[0:64, p, :], pctx_h[0][0:64, :])
        nc.vector.tensor_copy(ctx[64:128, p, :], pctx_h[1][64:128, :])

    # ---------------- Phase O: output projection -------------------
    if "out" not in phases:
        return
    if use_rs:
        dp_cm = tc.tile_pool(name=f"dram{rep}", bufs=1, space="DRAM")
        dp = dp_cm.__enter__()
        obounce = dp.tile([T, D], F32, tag="ob")
        ors = dp.tile([T // 2, D], F32, tag="ors")
    for tt in range(T // P):
        pso = psb.tile([P, 1024], F32, tag="big")
        for m in range(NMROW):
            for n in range(NT2):
                nc.tensor.matmul(
                    pso[:, n * 512:(n + 1) * 512],
                    lhsT=ctx[:, m, tt * P:(tt + 1) * P],
                    rhs=wo_t[:, m, n * 512:(n + 1) * 512],
                    start=(m == 0), stop=(m == NMROW - 1))
        osb = op_.tile([P, D], F32, tag="o")
        nc.vector.tensor_copy(osb[:], pso[:])
        dst = obounce if use_rs else out
        nc.sync.dma_start(dst[tt * P:(tt + 1) * P, :], osb[:])
    if use_rs:
        nc.gpsimd.collective_compute(
            "ReduceScatter",
            mybir.AluOpType.add,
            replica_groups=RG_PAIRS,
            ins=[obounce.opt()],
            outs=[ors.opt()],
        )
        nc.sync.dma_start(out[:], ors[:])
        dp_cm.__exit__(None, None, None)


def _build(maskout: bool, heads_per_core: int, use_rs: bool, repeat: int = 1,
           loop_reps: int = 0,
           phases=frozenset({"loads", "weights", "wvv", "proj", "heads", "out"})):
    """Build + compile the SPMD program. Returns compiled nc.

    loop_reps > 0 wraps the body in a tc.For_i hardware loop (no collectives
    allowed in that mode) — used only for differential timing."""
    HC = heads_per_core
    WCOLS = HC * DK
    OUT_ROWS = T // 2 if use_rs else T

    nc = bacc.Bacc("TRN2", target_bir_lowering=False, debug=False,
                   num_devices=N_CORES)

    qT = nc.dram_tensor("qT", [D, T], F32, kind="ExternalInput").ap()
    kT = nc.dram_tensor("kT", [D, T], F32, kind="ExternalInput").ap()
    vT = nc.dram_tensor("vT", [D, T], F32, kind="ExternalInput").ap()
    wq = nc.dram_tensor("wq", [D, WCOLS], F32, kind="ExternalInput").ap()
    wk = nc.dram_tensor("wk", [D, WCOLS], F32, kind="ExternalInput").ap()
    wv = nc.dram_tensor("wv", [D, WCOLS], F32, kind="ExternalInput").ap()
    wo = nc.dram_tensor("wo", [WCOLS, D], F32, kind="ExternalInput").ap()
    tri = nc.dram_tensor("tri", [P, WCOLS], F32, kind="ExternalInput").ap()
    ones = nc.dram_tensor("ones", [P, 1], F32, kind="ExternalInput").ap()
    out = nc.dram_tensor("out", [OUT_ROWS, D], F32, kind="ExternalOutput").ap()
    aps = (qT, kT, vT, wq, wk, wv, wo, tri, ones, out)

    with tile.TileContext(nc) as tc:
        with (
            tc.tile_pool(name="persist", bufs=1) as pp,
            tc.tile_pool(name="trans", bufs=4) as tp,
            tc.tile_pool(name="osb", bufs=2) as op_,
            tc.tile_pool(name="psum_big", bufs=2, space="PSUM") as psb,
            tc.tile_pool(name="psum_ctx", bufs=1, space="PSUM") as psc,
        ):
            if loop_reps:
                assert not use_rs, "collectives cannot live inside For_i"
                with tc.For_i(0, loop_reps, 1):
                    _emit_rep(nc, tc, aps, pp, tp, op_, psb, psc,
                              maskout, HC, use_rs, 0, phases=phases)
            else:
                for rep in range(repeat):
                    _emit_rep(nc, tc, aps, pp, tp, op_, psb, psc,
                              maskout, HC, use_rs, rep, phases=phases)

    nc.compile()
    nc.m = get_hw_module(nc.m)
    return nc


_CACHE: dict = {}


def _get_program(maskout: bool, heads_per_core: int, use_rs: bool,
                 repeat: int = 1):
    key = (maskout, heads_per_core, use_rs, repeat)
    if key not in _CACHE:
        _CACHE[key] = _build(*key)
    return _CACHE[key]


def _prep_inputs(Q, K, V, Wq, Wk, Wv, Wo, heads_per_core):
    """Host-side layout-only sharding: per-core input dicts."""
    HC = heads_per_core
    WCOLS = HC * DK
    tri = (np.arange(P)[:, None] >= (np.arange(WCOLS)[None, :] % DK)) \
        .astype(np.float32)
    ones = np.ones((P, 1), np.float32)
    in_maps = []
    for c in range(N_CORES):
        b = c // 2
        if HC == H:
            hsel = np.arange(H)
        else:
            g = c % 2
            hsel = np.arange(g * HC, (g + 1) * HC)
        # (H,D,DK) -> (D, HC*DK) packed columns for selected heads
        wq_p = np.ascontiguousarray(
            Wq[hsel].transpose(1, 0, 2).reshape(D, WCOLS))
        wk_p = np.ascontiguousarray(
            Wk[hsel].transpose(1, 0, 2).reshape(D, WCOLS))
        wv_p = np.ascontiguousarray(
            Wv[hsel].transpose(1, 0, 2).reshape(D, WCOLS))
        wo_p = np.ascontiguousarray(Wo.reshape(H, DK, D)[hsel].reshape(WCOLS, D))
        in_maps.append({
            "qT": np.ascontiguousarray(Q[b].T),
            "kT": np.ascontiguousarray(K[b].T),
            "vT": np.ascontiguousarray(V[b].T),
            "wq": wq_p, "wk": wk_p, "wv": wv_p, "wo": wo_p,
            "tri": tri, "ones": ones,
        })
    return in_maps


def run(Q, K, V, Wq, Wk, Wv, Wo, maskout, heads_per_core=8, use_rs=True,
        repeat=1):
    Q = np.asarray(Q, np.float32)
    K = np.asarray(K, np.float32)
    V = np.asarray(V, np.float32)
    Wq = np.asarray(Wq, np.float32)
    Wk = np.asarray(Wk, np.float32)
    Wv = np.asarray(Wv, np.float32)
    Wo = np.asarray(Wo, np.float32)
    mk = bool(np.asarray(maskout).item())
    nc = _get_program(mk, heads_per_core, use_rs, repeat)
    in_maps = _prep_inputs(Q, K, V, Wq, Wk, Wv, Wo, heads_per_core)
    res = bass_utils.run_bass_kernel_spmd(
        nc, in_maps, list(range(N_CORES)), trace=False)
    outf = np.empty((B, T, D), np.float32)
    for c in range(N_CORES):
        b = c // 2
        if use_rs:
            half = c % 2
            outf[b, half * (T // 2):(half + 1) * (T // 2), :] = \
                res.results[c]["out"]
        else:
            if c % 2 == 0:
                outf[b] = res.results[c]["out"]
    return outf, res


def kernel(Q, K, V, Wq, Wk, Wv, Wo, maskout):
    outf, _ = run(Q, K, V, Wq, Wk, Wv, Wo, maskout,
                  heads_per_core=8, use_rs=True)
    return outf
